# revision 5
# baseline (speedup 1.0000x reference)
"""Negative cross-correlation loss: out = -sum(x * y).

Full inputs x, y: (16, 4000, 512, 1) f32. Data-parallel over the shot axis:
2 shots per core on 8 NeuronCores. Each core DMAs its 2x4000x512 shard as
8 tiles of [128, 4000], fuses multiply+per-partition-reduce on the vector
engine (scalar_tensor_tensor accum_out), reduces across tiles and
partitions, and writes one scalar. Host sums the 8 partials and negates.
"""

import numpy as np

import jax
from jax.experimental.shard_map import shard_map
from jax.sharding import Mesh, NamedSharding, PartitionSpec

import concourse.bacc as bacc
import concourse.mybir as mybir
import concourse.tile as tile
from concourse import bass2jax
from concourse.bass_isa import ReduceOp

N_CORES = 8
P = 128
# Per-core shard: 2 shots * 4000 * 512 * 1 = 4_096_000 f32 elements.
SHARD_ELEMS = 2 * 4000 * 512
TILE_W = 4000
N_TILES = SHARD_ELEMS // (P * TILE_W)  # 8
assert N_TILES * P * TILE_W == SHARD_ELEMS


def _build_nc(repeat=1):
    """Bass kernel for one core. `repeat` re-runs the identical body that many
    times (same data, same result) — used only for wall-clock slope timing."""
    nc = bacc.Bacc("TRN2", target_bir_lowering=False, debug=False)
    x = nc.dram_tensor("x", [N_TILES * P, TILE_W], mybir.dt.float32, kind="ExternalInput")
    y = nc.dram_tensor("y", [N_TILES * P, TILE_W], mybir.dt.float32, kind="ExternalInput")
    out = nc.dram_tensor("out", [1, 1], mybir.dt.float32, kind="ExternalOutput")

    xa, ya, oa = x.ap(), y.ap(), out.ap()

    with tile.TileContext(nc) as tc:
        with (
            tc.tile_pool(name="io", bufs=6) as io_pool,
            tc.tile_pool(name="red", bufs=1) as red_pool,
        ):
            acc = red_pool.tile([P, N_TILES], mybir.dt.float32)
            dummy = red_pool.tile([P, 1], mybir.dt.float32)
            for _ in range(repeat):
                for t in range(N_TILES):
                    xt = io_pool.tile([P, TILE_W], mybir.dt.float32, tag="xt")
                    yt = io_pool.tile([P, TILE_W], mybir.dt.float32, tag="yt")
                    nc.sync.dma_start(out=xt[:], in_=xa[t * P : (t + 1) * P, :])
                    nc.sync.dma_start(out=yt[:], in_=ya[t * P : (t + 1) * P, :])
                    # acc[:, t] = sum_w xt*yt (per-partition); dummy absorbs
                    # the elementwise product via a stride-0 output.
                    nc.vector.scalar_tensor_tensor(
                        out=dummy.broadcast_to(xt.shape),
                        in0=xt[:],
                        scalar=1.0,
                        in1=yt[:],
                        op0=mybir.AluOpType.mult,
                        op1=mybir.AluOpType.mult,
                        accum_out=acc[:, t : t + 1],
                    )
            total = red_pool.tile([P, 1], mybir.dt.float32)
            nc.vector.tensor_reduce(
                out=total[:],
                in_=acc[:],
                axis=mybir.AxisListType.X,
                op=mybir.AluOpType.add,
            )
            nc.gpsimd.partition_all_reduce(total[:], total[:], P, ReduceOp.add)
            nc.sync.dma_start(out=oa[:, :], in_=total[:1, :1])

    nc.compile()
    return nc


class Runner:
    """Compiles the per-core Bass kernel once and keeps a cached jitted
    shard_map executable over 8 cores (mirrors bass2jax.run_bass_via_pjrt's
    multi-core path, minus the per-call retrace and host concat)."""

    def __init__(self, repeat=1):
        bass2jax.install_neuronx_cc_hook()
        nc = _build_nc(repeat)
        self.nc = nc

        in_names = ["x", "y"]
        out_names = ["out"]
        out_avals = (jax.core.ShapedArray((1, 1), np.float32),)
        all_in_names = tuple(in_names + out_names + [nc.partition_id_tensor.name])

        def _body(*args):
            outs = bass2jax._bass_exec_p.bind(
                *args,
                bass2jax.partition_id_tensor(),
                out_avals=out_avals,
                in_names=all_in_names,
                out_names=tuple(out_names),
                lowering_input_output_aliases=(),
                sim_require_finite=True,
                sim_require_nnan=True,
                nc=nc,
            )
            return tuple(outs)

        devices = jax.devices()[:N_CORES]
        assert len(devices) == N_CORES
        self.mesh = Mesh(np.asarray(devices), ("core",))
        self.sharding = NamedSharding(self.mesh, PartitionSpec("core"))
        in_specs = (PartitionSpec("core"),) * 3
        out_specs = (PartitionSpec("core"),)
        self.fn = jax.jit(
            shard_map(
                _body,
                mesh=self.mesh,
                in_specs=in_specs,
                out_specs=out_specs,
                check_rep=False,
            ),
            donate_argnums=(2,),
            keep_unused=True,
        )

    def __call__(self, x_all, y_all):
        """x_all, y_all: [N_CORES * N_TILES * P, TILE_W] f32 (host or device).
        Returns the 8 per-core partial sums as a host np array."""
        zeros = np.zeros((N_CORES, 1), np.float32)
        (out,) = self.fn(x_all, y_all, zeros)
        return np.asarray(out).reshape(N_CORES)


_RUNNER = None


def _get_runner():
    global _RUNNER
    if _RUNNER is None:
        _RUNNER = Runner()
    return _RUNNER


def kernel(x, y, win=None, step=None):
    # Row-block c of the reshaped [8192, TILE_W] array is exactly core c's
    # shard (shots 2c, 2c+1) — shard_map's axis-0 split does the sharding.
    x = np.ascontiguousarray(np.asarray(x, dtype=np.float32)).reshape(
        N_CORES * N_TILES * P, TILE_W
    )
    y = np.ascontiguousarray(np.asarray(y, dtype=np.float32)).reshape(
        N_CORES * N_TILES * P, TILE_W
    )
    parts = _get_runner()(x, y)
    return np.float32(-np.float64(parts.sum()))
